# revision 16
# baseline (speedup 1.0000x reference)
"""Sliding-window (banded causal) multi-head attention on 8 TRN2 NeuronCores.

Sharding: 8 cores = 2 batches x 4 head-groups (4 heads of 64 dims each).
Each core computes QKV projections for its 4 heads, RoPE, banded flash
attention (window 1024), and a partial output projection (its 256 columns
of wo). The host sums the 4 partial outputs per batch element.

Schedule (v2), built around three cost-model facts: matmul time is
out-free-size x 0.4167ns with contraction/partitions free, the PE ramps
to full clock only after 3us of continuous execution, and exp on the
Activation engine (~0.83ns/col + 185ns/instr) is the attention-phase
co-bottleneck with ~58us of mandatory work.

  - warmup matmuls on the masks tile keep the PE busy from ~0.4us so the
    p-state is fully ramped when the real projections start
  - phases are interleaved so Activation's exp stream starts early:
    QK proj (m=0) -> V tiles 0-3 -> attention pass A (mt=0) with the
    remaining V tiles and the m=1 QK projection woven in as PE filler,
    -> attention pass B (mt=1, descending qi) with the output projection
    and stores trailing two tiles behind
  - ctx matmuls use probs as the stationary operand: out [q,65] per
    (head, k-tile) costs 65 columns instead of 128 (the [65,q]
    orientation wastes half the PE partitions); a PE transpose per
    (mt, q-tile) restores the [dims, tokens] layout for the out proj
  - softmax denominators ride a ones-column in V; normalization is two
    per-partition scalar multiplies on DVE; biases ride the RoPE muls
  - pass B runs q-tiles descending so the tail drains through tile 0,
    whose 1-block attention chain is the shortest
"""

from contextlib import ExitStack

import numpy as np
import ml_dtypes

import concourse.bass as bass
import concourse.tile as tile
from concourse import bacc, mybir
from concourse.bass_utils import run_bass_kernel_spmd

BF16 = mybir.dt.bfloat16
F32 = mybir.dt.float32

B, S, H = 2, 2048, 1024
NH, HD = 16, 64
WINDOW = 1024
ROPE_THETA = 10000.0
MAX_POS = 2048
N_CORES = 8
HG = 4                      # heads per core
GD = HG * HD                # 256: head-group dim per core
P = 128
NQT = S // P                # 16 q tiles
WT = WINDOW // P            # 8: window in tiles
CH = H // P                 # 8 contraction chunks
VW = HD + 1                 # 65: V width with ones column
K_WARM = 10                 # PE warmup matmuls (p-state ramp during DMA)

_cache = {}


def _build():
    nc = bacc.Bacc("TRN2", target_bir_lowering=False, debug=False,
                   enable_asserts=False, num_devices=N_CORES)

    xT_d = nc.dram_tensor("xT", [H, S], BF16, kind="ExternalInput")
    wqT_d = nc.dram_tensor("wqT", [H, GD], BF16, kind="ExternalInput")
    wkT_d = nc.dram_tensor("wkT", [H, GD], BF16, kind="ExternalInput")
    wvT_d = nc.dram_tensor("wvT", [H, GD], BF16, kind="ExternalInput")
    woT_d = nc.dram_tensor("woT", [GD, H], BF16, kind="ExternalInput")
    cosT_d = nc.dram_tensor("cosT", [P, S], BF16, kind="ExternalInput")
    sinTs_d = nc.dram_tensor("sinTs", [P, S], BF16, kind="ExternalInput")
    bq_d = nc.dram_tensor("bq2", [P, 2], F32, kind="ExternalInput")
    bk_d = nc.dram_tensor("bk2", [P, 2], F32, kind="ExternalInput")
    bqs_d = nc.dram_tensor("bq2s", [P, 2], F32, kind="ExternalInput")
    bks_d = nc.dram_tensor("bk2s", [P, 2], F32, kind="ExternalInput")
    # [diag mask | far mask | identity], bf16
    consts_d = nc.dram_tensor("consts", [P, 3 * P], BF16, kind="ExternalInput")
    out_d = nc.dram_tensor("out", [S, H], F32, kind="ExternalOutput")

    with tile.TileContext(nc) as tc, ExitStack() as ctx:
        const = ctx.enter_context(tc.tile_pool(name="const", bufs=1))
        qk = ctx.enter_context(tc.tile_pool(name="qk", bufs=1))
        vp = ctx.enter_context(tc.tile_pool(name="vp", bufs=1))
        pb = ctx.enter_context(tc.tile_pool(name="pb", bufs=4))
        nr = ctx.enter_context(tc.tile_pool(name="nr", bufs=3))
        cxs = ctx.enter_context(tc.tile_pool(name="cxs", bufs=1))
        osb = ctx.enter_context(tc.tile_pool(name="osb", bufs=4))

        wq_sb = const.tile([P, CH * GD], BF16, name="wq_sb")
        wk_sb = const.tile([P, CH * GD], BF16, name="wk_sb")
        wv_sb = const.tile([P, CH * GD], BF16, name="wv_sb")
        wo_sb = const.tile([P, 2 * H], BF16, name="wo_sb")
        x_sb = const.tile([P, CH * S], BF16, name="x_sb")
        cosT = const.tile([P, S], BF16, name="cosT")
        sinTs = const.tile([P, S], BF16, name="sinTs")
        bq_sb = const.tile([P, 2], F32, name="bq_sb")
        bk_sb = const.tile([P, 2], F32, name="bk_sb")
        bqs_sb = const.tile([P, 2], F32, name="bqs_sb")
        bks_sb = const.tile([P, 2], F32, name="bks_sb")
        consts = const.tile([P, 3 * P], BF16, name="consts")

        def chunked(dram, w):
            return dram.ap().rearrange("(c p) w -> p c w", p=P)

        # x chunks alternate between the two queues so the arrival cadence
        # (~1.3us/chunk effective) outpaces the m=0 projection's consumption
        xv = x_sb.rearrange("p (c w) -> p c w", c=CH)
        xs = chunked(xT_d, S)
        nc.sync.dma_start(consts[:], consts_d.ap())
        nc.scalar.dma_start(wq_sb.rearrange("p (c w) -> p c w", c=CH),
                            chunked(wqT_d, GD))
        nc.scalar.dma_start(wk_sb.rearrange("p (c w) -> p c w", c=CH),
                            chunked(wkT_d, GD))
        for c in range(0, CH, 2):
            nc.sync.dma_start(xv[:, c:c + 1], xs[:, c:c + 1])
        for c in range(1, CH, 2):
            nc.scalar.dma_start(xv[:, c:c + 1], xs[:, c:c + 1])
        nc.scalar.dma_start(wv_sb.rearrange("p (c w) -> p c w", c=CH),
                            chunked(wvT_d, GD))
        nc.sync.dma_start(cosT[:], cosT_d.ap())
        nc.sync.dma_start(sinTs[:], sinTs_d.ap())
        nc.sync.dma_start(bq_sb[:], bq_d.ap())
        nc.sync.dma_start(bk_sb[:], bk_d.ap())
        nc.sync.dma_start(bqs_sb[:], bqs_d.ap())
        nc.sync.dma_start(bks_sb[:], bks_d.ap())
        nc.scalar.dma_start(wo_sb.rearrange("p (c w) -> p c w", c=2),
                            chunked(woT_d, H))

        def xc(c):
            return x_sb[:, c * S:(c + 1) * S]

        def wc(w_sb, c, width=GD):
            return w_sb[:, c * width:(c + 1) * width]

        q_sb = [qk.tile([P, S], BF16, name=f"q{m}") for m in range(2)]
        k_sb = [qk.tile([P, S], BF16, name=f"k{m}") for m in range(2)]
        qs_sb = [qk.tile([P, S], BF16, name=f"qs{m}") for m in range(2)]
        ks_sb = [qk.tile([P, S], BF16, name=f"ks{m}") for m in range(2)]
        v_sb = [vp.tile([P, HG * VW], BF16, name=f"v{t}") for t in range(NQT)]
        ctx_sb = [cxs.tile([P, S], BF16, name=f"cx{m}") for m in range(2)]
        _osb = [osb.tile([P, H], F32, tag="osb", name=f"ot{t}", bufs=4)
                for t in range(NQT)]

        # ones columns for the softmax denominators (Pool, idle early)
        for t in range(NQT):
            nc.gpsimd.memset(v_sb[t][:], 1.0)

        def rope_piece(m, lo, hi, src, shf, bc, bs, eng):
            cl = slice(lo, hi)
            for hb in range(2):
                o = hb * HD
                nc.gpsimd.dma_start(shf[m][o:o + 32, cl],
                                    src[m][o + 32:o + 64, cl])
                nc.gpsimd.dma_start(shf[m][o + 32:o + 64, cl],
                                    src[m][o:o + 32, cl])
            eng.scalar_tensor_tensor(
                shf[m][:, cl], shf[m][:, cl], bs[:, m:m + 1], sinTs[:, cl],
                mybir.AluOpType.add, mybir.AluOpType.mult)
            eng.scalar_tensor_tensor(
                src[m][:, cl], src[m][:, cl], bc[:, m:m + 1], cosT[:, cl],
                mybir.AluOpType.add, mybir.AluOpType.mult)
            eng.tensor_add(src[m][:, cl], src[m][:, cl], shf[m][:, cl])

        # PSUM is hand-laid-out in two persistent tiles (region-granular
        # dependency tracking): spt = scores for one q-tile, both heads;
        # cpt = ctx accumulators + transpose slot + filler/outproj banks.
        SH = WT * P + P  # 1152: score columns per head

        def attn_scores(spt, mt, qi):
            kt0 = max(0, qi - WT)
            # slot order: [diag, far?, middles...]
            kts = [qi]
            n_edge = 1
            if qi >= WT:
                kts.append(kt0)
                n_edge = 2
            kts.extend(range(kt0 + (1 if qi >= WT else 0), qi))
            nkt = len(kts)
            prb = []
            for hb in range(2):
                ho = hb * HD
                s_ps = spt[:, hb * SH:hb * SH + SH]
                for i, kt in enumerate(kts):
                    nc.tensor.matmul(
                        s_ps[:, i * P:(i + 1) * P],
                        k_sb[mt][ho:ho + HD, kt * P:(kt + 1) * P],
                        q_sb[mt][ho:ho + HD, qi * P:(qi + 1) * P],
                        start=True, stop=True)
                probs = pb.tile([P, SH], BF16, tag="pb",
                                name=f"pr{mt}{qi}{hb}")
                nc.scalar.activation(
                    probs[:, 0:nkt * P], s_ps[:, 0:nkt * P],
                    mybir.ActivationFunctionType.Exp,
                    scale=float(1.0 / np.sqrt(HD)))
                # edge masks on Pool: DVE is near-saturated in pass B
                nc.gpsimd.tensor_mul(
                    probs[:, 0:n_edge * P], probs[:, 0:n_edge * P],
                    consts[:, 0:n_edge * P])
                prb.append(probs)
            return kts, n_edge, prb

        def attn_ctx(cxp, mt, qi, kts, n_edge, prb):
            nkt = len(kts)
            issue = list(range(n_edge, nkt)) + list(range(n_edge))
            for hb in range(2):
                h = mt * 2 + hb
                for j, i in enumerate(issue):
                    nc.tensor.matmul(
                        cxp[:, hb * VW:(hb + 1) * VW],
                        prb[hb][:, i * P:(i + 1) * P],
                        v_sb[kts[i]][:, h * VW:(h + 1) * VW],
                        start=(j == 0), stop=(j == nkt - 1))

        def norm_t(cxp, tpv, mt, qi):
            rcp = nr.tile([P, 2], F32, tag="rcp", name=f"rc{mt}{qi}")
            nrm = nr.tile([P, P], BF16, tag="nrm", name=f"nm{mt}{qi}")
            cxv = cxp.rearrange("p (h w) -> p h w", h=2)
            nc.vector.reciprocal(rcp[:], cxv[:, :, HD])
            for hb in range(2):
                nc.vector.tensor_scalar_mul(
                    nrm[:, hb * HD:(hb + 1) * HD], cxv[:, hb, 0:HD],
                    rcp[:, hb:hb + 1])
            nc.tensor.transpose(tpv, nrm[:], consts[:, 2 * P:3 * P])
            nc.vector.tensor_copy(ctx_sb[mt][:, qi * P:(qi + 1) * P], tpv)

        def vchain(ps, t):
            for c in range(CH):
                nc.tensor.matmul(ps[:, 0:GD], xc(c)[:, t * P:(t + 1) * P],
                                 wc(wv_sb, c), start=(c == 0), stop=(c == CH - 1))
            vdst = v_sb[t].rearrange("p (h d) -> p h d", h=HG)[:, :, 0:HD]
            vsrc = ps[:, 0:GD].rearrange("p (h d) -> p h d", h=HG)
            nc.scalar.copy(vdst, vsrc)

        def m1chain(ps, w_sb, dest, n):
            for c in range(CH):
                nc.tensor.matmul(
                    ps[:], wc(w_sb, c)[:, P:2 * P],
                    xc(c)[:, n * 512:(n + 1) * 512],
                    start=(c == 0), stop=(c == CH - 1))
            nc.vector.tensor_copy(dest[1][:, n * 512:(n + 1) * 512], ps[:])

        def outproj_t(fla, flb, t, last=False):
            # GPSIMD has no PSUM port and DMA can't read PSUM: drains on DVE.
            o_sb = _osb[t]
            for n, ps in enumerate((fla, flb)):
                for c in range(2):
                    nc.tensor.matmul(
                        ps[:], ctx_sb[c][:, t * P:(t + 1) * P],
                        wc(wo_sb, c, H)[:, n * 512:(n + 1) * 512],
                        start=(c == 0), stop=(c == 1))
                nc.vector.tensor_copy(o_sb[:, n * 512:(n + 1) * 512], ps[:])
                if last:
                    # halves stream out on both idle queues as soon as staged
                    e = nc.gpsimd if n == 0 else nc.sync
                    e.dma_start(out_d.ap()[t * P:(t + 1) * P,
                                           n * 512:(n + 1) * 512],
                                o_sb[:, n * 512:(n + 1) * 512])
            if not last:
                # keep store triggers off the Act queue: an in-order trigger
                # waiting on its staging buffer would stall the exps
                e = nc.gpsimd if t % 2 == 0 else nc.sync
                e.dma_start(out_d.ap()[t * P:(t + 1) * P, :], o_sb[:])

        # ---- warmup + QK m=0 projection ----
        with tc.tile_pool(name="pj", bufs=8, space="PSUM") as pj:
            # warmup borrows a pj buffer (freed by the WAR dep on bank reuse);
            # one long accumulation chain so no intermediate sems serialize it
            warm = pj.tile([P, 512], F32, tag="pj", name="warm")
            for i in range(K_WARM):
                nc.tensor.matmul(warm[:, 0:3 * P], consts[:, 0:P], consts[:],
                                 start=(i == 0), stop=(i == K_WARM - 1))
            pss = [[pj.tile([P, 512], F32, tag="pj", name=f"pj{i}{n}")
                    for n in range(4)] for i in range(2)]
            for c in range(CH):
                for i, w in enumerate((wq_sb, wk_sb)):
                    for n in range(4):
                        nc.tensor.matmul(
                            pss[i][n][:], wc(w, c)[:, 0:P],
                            xc(c)[:, n * 512:(n + 1) * 512],
                            start=(c == 0), stop=(c == CH - 1))
            # drain + rope quarter-by-quarter so attention can start after
            # the first quarter; q rope on DVE, k rope on Pool (parallel)
            for n in range(4):
                nc.scalar.copy(q_sb[0][:, n * 512:(n + 1) * 512],
                               pss[0][n][:])
                nc.scalar.copy(k_sb[0][:, n * 512:(n + 1) * 512],
                               pss[1][n][:])
                rope_piece(0, n * 512, (n + 1) * 512, q_sb, qs_sb,
                           bq_sb, bqs_sb, nc.vector)
                rope_piece(0, n * 512, (n + 1) * 512, k_sb, ks_sb,
                           bk_sb, bks_sb, nc.gpsimd)

        # ---- main phase ----
        # psum: spt 2304 f32 (5 banks) + cpt 1536 f32 (3 banks) = 8 banks
        with tc.tile_pool(name="sp", bufs=1, space="PSUM") as sp_pool, \
             tc.tile_pool(name="cp", bufs=1, space="PSUM") as cp_pool:
            spt = sp_pool.tile([P, 2 * SH], F32, name="spt")
            cpt = cp_pool.tile([P, 1536], F32, name="cpt")
            CTX = cpt[:, 0:2 * VW]
            TPV = cpt[:, 2 * VW:2 * VW + HD].bitcast(BF16)    # [128, 128]
            FLA = cpt[:, 194:706]
            FLB = cpt[:, 706:1218]
            FLC = cpt[:, 1218:1474]

            # V tiles 0-3 before pass A (ctx qi needs V k-tiles <= qi)
            vchain(FLC, 0)
            vchain(FLA[:, 0:GD], 1)
            vchain(FLB[:, 0:GD], 2)
            vchain(FLC, 3)

            # m=1 chain order feeds rope(1) halves as early as possible
            m1args = [(wq_sb, q_sb, 0), (wq_sb, q_sb, 1),
                      (wk_sb, k_sb, 0), (wk_sb, k_sb, 1),
                      (wq_sb, q_sb, 2), (wq_sb, q_sb, 3),
                      (wk_sb, k_sb, 2), (wk_sb, k_sb, 3)]
            rope1 = {1: (q_sb, qs_sb, bq_sb, bqs_sb, 0),
                     3: (k_sb, ks_sb, bk_sb, bks_sb, 0),
                     5: (q_sb, qs_sb, bq_sb, bqs_sb, 1),
                     7: (k_sb, ks_sb, bk_sb, bks_sb, 1)}

            # ---- pass A (mt=0), ascending ----
            pend = None
            for qi in range(NQT):
                cur = attn_scores(spt, 0, qi)
                if pend is not None:
                    attn_ctx(CTX, 0, qi - 1, *pend)
                    norm_t(CTX, TPV, 0, qi - 1)
                pend = cur
                # PE filler work, emitted after the critical attention ops
                if qi + 4 < NQT:
                    vchain(FLC, qi + 4)
                if qi <= 7:
                    w_sb_, dest_, n_ = m1args[qi]
                    m1chain(FLA if qi % 2 == 0 else FLB, w_sb_, dest_, n_)
                if qi in rope1:
                    src_, shf_, bc_, bs_, half_ = rope1[qi]
                    for half2 in range(2):
                        lo = half_ * 1024 + half2 * 512
                        rope_piece(1, lo, lo + 512, src_, shf_, bc_, bs_,
                                   nc.vector)
            attn_ctx(CTX, 0, NQT - 1, *pend)
            norm_t(CTX, TPV, 0, NQT - 1)

            # ---- pass B (mt=1), descending; outproj trails ----
            pend = None
            for qi in range(NQT - 1, -1, -1):
                cur = attn_scores(spt, 1, qi)
                if pend is not None:
                    attn_ctx(CTX, 1, qi + 1, *pend)
                    norm_t(CTX, TPV, 1, qi + 1)
                if qi + 2 < NQT - 1:
                    outproj_t(FLA, FLB, qi + 3)
                pend = cur
            attn_ctx(CTX, 1, 0, *pend)
            norm_t(CTX, TPV, 1, 0)
            outproj_t(FLA, FLB, 2)
            outproj_t(FLA, FLB, 1, last=True)
            outproj_t(FLA, FLB, 0, last=True)

    nc.compile()
    return nc


def _rope_tables():
    inv_freq = 1.0 / (ROPE_THETA ** (np.arange(0, HD, 2, dtype=np.float64) / HD))
    t = np.arange(MAX_POS, dtype=np.float64)
    freqs = np.outer(t, inv_freq)                       # [MAX_POS, 32]
    emb = np.concatenate([freqs, freqs], axis=-1)       # [MAX_POS, 64]
    return np.cos(emb).astype(np.float32), np.sin(emb).astype(np.float32)


def kernel(hidden_states, position_ids, wq, bq, wk, bk, wv, bv, wo, bo):
    bf16 = ml_dtypes.bfloat16
    if "nc" not in _cache:
        _cache["nc"] = _build()
    nc = _cache["nc"]

    cos_t, sin_t = _rope_tables()
    pos = np.clip(np.asarray(position_ids), 0, MAX_POS - 1).astype(np.int64)

    maskd = np.triu(np.ones((P, P), np.float32))        # k <= q (diag block)
    maskf = np.tril(np.ones((P, P), np.float32), -1)    # k > q  (far block)
    ident = np.eye(P, dtype=np.float32)
    consts = np.concatenate([maskd, maskf, ident], axis=1).astype(bf16)

    in_maps = []
    for core in range(N_CORES):
        b, g = core // HG, core % HG
        sl = slice(g * GD, (g + 1) * GD)
        cos_b = cos_t[pos[b]]                            # [S, 64]
        sin_b = sin_t[pos[b]]
        cosT = np.tile(cos_b.T, (2, 1)).astype(bf16)     # [128, S]
        sin_sgn = sin_b.T.copy()                         # [64, S]
        sin_sgn[0:32] *= -1.0
        sinTs = np.tile(sin_sgn, (2, 1)).astype(bf16)
        in_maps.append({
            "xT": np.ascontiguousarray(hidden_states[b].T).astype(bf16),
            "wqT": np.ascontiguousarray(wq[sl].T).astype(bf16),
            "wkT": np.ascontiguousarray(wk[sl].T).astype(bf16),
            "wvT": np.ascontiguousarray(wv[sl].T).astype(bf16),
            "woT": np.ascontiguousarray(wo[:, sl].T).astype(bf16),
            "cosT": cosT,
            "sinTs": sinTs,
            "bq2": np.ascontiguousarray(
                bq[sl].reshape(2, P).T).astype(np.float32),
            "bk2": np.ascontiguousarray(
                bk[sl].reshape(2, P).T).astype(np.float32),
            "bq2s": np.ascontiguousarray(
                bq[sl].reshape(2, 2, 2, 32)[:, :, ::-1].reshape(
                    2, P).T).astype(np.float32),
            "bk2s": np.ascontiguousarray(
                bk[sl].reshape(2, 2, 2, 32)[:, :, ::-1].reshape(
                    2, P).T).astype(np.float32),
            "consts": consts,
        })

    res = run_bass_kernel_spmd(nc, in_maps, core_ids=list(range(N_CORES)))

    const_off = (wo @ bv + bo).astype(np.float32)        # host-folded biases
    out = np.empty((B, S, H), dtype=np.float32)
    for b in range(B):
        acc = res.results[b * HG]["out"].astype(np.float32).copy()
        for g in range(1, HG):
            acc += res.results[b * HG + g]["out"]
        out[b] = acc + const_off[None, :]
    return out


# revision 17
# speedup vs baseline: 1.0002x; 1.0002x over previous
"""Sliding-window (banded causal) multi-head attention on 8 TRN2 NeuronCores.

Sharding: 8 cores = 2 batches x 4 head-groups (4 heads of 64 dims each).
Each core computes QKV projections for its 4 heads, RoPE, banded flash
attention (window 1024), and a partial output projection (its 256 columns
of wo). The host sums the 4 partial outputs per batch element.

Schedule (v2), built around three cost-model facts: matmul time is
out-free-size x 0.4167ns with contraction/partitions free, the PE ramps
to full clock only after 3us of continuous execution, and exp on the
Activation engine (~0.83ns/col + 185ns/instr) is the attention-phase
co-bottleneck with ~58us of mandatory work.

  - warmup matmuls on the masks tile keep the PE busy from ~0.4us so the
    p-state is fully ramped when the real projections start
  - phases are interleaved so Activation's exp stream starts early:
    QK proj (m=0) -> V tiles 0-3 -> attention pass A (mt=0) with the
    remaining V tiles and the m=1 QK projection woven in as PE filler,
    -> attention pass B (mt=1, descending qi) with the output projection
    and stores trailing two tiles behind
  - ctx matmuls use probs as the stationary operand: out [q,65] per
    (head, k-tile) costs 65 columns instead of 128 (the [65,q]
    orientation wastes half the PE partitions); a PE transpose per
    (mt, q-tile) restores the [dims, tokens] layout for the out proj
  - softmax denominators ride a ones-column in V; normalization is two
    per-partition scalar multiplies on DVE; biases ride the RoPE muls
  - pass B runs q-tiles descending so the tail drains through tile 0,
    whose 1-block attention chain is the shortest
"""

from contextlib import ExitStack

import numpy as np
import ml_dtypes

import concourse.bass as bass
import concourse.tile as tile
from concourse import bacc, mybir
from concourse.bass_utils import run_bass_kernel_spmd

BF16 = mybir.dt.bfloat16
F32 = mybir.dt.float32

B, S, H = 2, 2048, 1024
NH, HD = 16, 64
WINDOW = 1024
ROPE_THETA = 10000.0
MAX_POS = 2048
N_CORES = 8
HG = 4                      # heads per core
GD = HG * HD                # 256: head-group dim per core
P = 128
NQT = S // P                # 16 q tiles
WT = WINDOW // P            # 8: window in tiles
CH = H // P                 # 8 contraction chunks
VW = HD + 1                 # 65: V width with ones column
K_WARM = 10                 # PE warmup matmuls (p-state ramp during DMA)

_cache = {}


def _build():
    nc = bacc.Bacc("TRN2", target_bir_lowering=False, debug=False,
                   enable_asserts=False, num_devices=N_CORES)

    xT_d = nc.dram_tensor("xT", [H, S], BF16, kind="ExternalInput")
    wqT_d = nc.dram_tensor("wqT", [H, GD], BF16, kind="ExternalInput")
    wkT_d = nc.dram_tensor("wkT", [H, GD], BF16, kind="ExternalInput")
    wvT_d = nc.dram_tensor("wvT", [H, GD], BF16, kind="ExternalInput")
    woT_d = nc.dram_tensor("woT", [GD, H], BF16, kind="ExternalInput")
    cosT_d = nc.dram_tensor("cosT", [P, S], BF16, kind="ExternalInput")
    sinTs_d = nc.dram_tensor("sinTs", [P, S], BF16, kind="ExternalInput")
    bq_d = nc.dram_tensor("bq2", [P, 2], F32, kind="ExternalInput")
    bk_d = nc.dram_tensor("bk2", [P, 2], F32, kind="ExternalInput")
    bqs_d = nc.dram_tensor("bq2s", [P, 2], F32, kind="ExternalInput")
    bks_d = nc.dram_tensor("bk2s", [P, 2], F32, kind="ExternalInput")
    # [diag mask | far mask | identity], bf16
    consts_d = nc.dram_tensor("consts", [P, 3 * P], BF16, kind="ExternalInput")
    out_d = nc.dram_tensor("out", [S, H], F32, kind="ExternalOutput")

    with tile.TileContext(nc) as tc, ExitStack() as ctx:
        const = ctx.enter_context(tc.tile_pool(name="const", bufs=1))
        qk = ctx.enter_context(tc.tile_pool(name="qk", bufs=1))
        vp = ctx.enter_context(tc.tile_pool(name="vp", bufs=1))
        pb = ctx.enter_context(tc.tile_pool(name="pb", bufs=4))
        nr = ctx.enter_context(tc.tile_pool(name="nr", bufs=3))
        cxs = ctx.enter_context(tc.tile_pool(name="cxs", bufs=1))
        osb = ctx.enter_context(tc.tile_pool(name="osb", bufs=4))

        wq_sb = const.tile([P, CH * GD], BF16, name="wq_sb")
        wk_sb = const.tile([P, CH * GD], BF16, name="wk_sb")
        wv_sb = const.tile([P, CH * GD], BF16, name="wv_sb")
        wo_sb = const.tile([P, 2 * H], BF16, name="wo_sb")
        x_sb = const.tile([P, CH * S], BF16, name="x_sb")
        cosT = const.tile([P, S], BF16, name="cosT")
        sinTs = const.tile([P, S], BF16, name="sinTs")
        bq_sb = const.tile([P, 2], F32, name="bq_sb")
        bk_sb = const.tile([P, 2], F32, name="bk_sb")
        bqs_sb = const.tile([P, 2], F32, name="bqs_sb")
        bks_sb = const.tile([P, 2], F32, name="bks_sb")
        consts = const.tile([P, 3 * P], BF16, name="consts")

        def chunked(dram, w):
            return dram.ap().rearrange("(c p) w -> p c w", p=P)

        # x chunks alternate between the two queues so the arrival cadence
        # (~1.3us/chunk effective) outpaces the m=0 projection's consumption
        xv = x_sb.rearrange("p (c w) -> p c w", c=CH)
        xs = chunked(xT_d, S)
        nc.sync.dma_start(consts[:], consts_d.ap())
        nc.scalar.dma_start(wq_sb.rearrange("p (c w) -> p c w", c=CH),
                            chunked(wqT_d, GD))
        nc.scalar.dma_start(wk_sb.rearrange("p (c w) -> p c w", c=CH),
                            chunked(wkT_d, GD))
        for c in range(0, CH, 2):
            nc.sync.dma_start(xv[:, c:c + 1], xs[:, c:c + 1])
        for c in range(1, CH, 2):
            nc.scalar.dma_start(xv[:, c:c + 1], xs[:, c:c + 1])
        nc.scalar.dma_start(wv_sb.rearrange("p (c w) -> p c w", c=CH),
                            chunked(wvT_d, GD))
        nc.sync.dma_start(cosT[:], cosT_d.ap())
        nc.sync.dma_start(sinTs[:], sinTs_d.ap())
        nc.sync.dma_start(bq_sb[:], bq_d.ap())
        nc.sync.dma_start(bk_sb[:], bk_d.ap())
        nc.sync.dma_start(bqs_sb[:], bqs_d.ap())
        nc.sync.dma_start(bks_sb[:], bks_d.ap())
        nc.scalar.dma_start(wo_sb.rearrange("p (c w) -> p c w", c=2),
                            chunked(woT_d, H))

        def xc(c):
            return x_sb[:, c * S:(c + 1) * S]

        def wc(w_sb, c, width=GD):
            return w_sb[:, c * width:(c + 1) * width]

        q_sb = [qk.tile([P, S], BF16, name=f"q{m}") for m in range(2)]
        k_sb = [qk.tile([P, S], BF16, name=f"k{m}") for m in range(2)]
        qs_sb = [qk.tile([P, S], BF16, name=f"qs{m}") for m in range(2)]
        ks_sb = [qk.tile([P, S], BF16, name=f"ks{m}") for m in range(2)]
        v_sb = [vp.tile([P, HG * VW], BF16, name=f"v{t}") for t in range(NQT)]
        ctx_sb = [cxs.tile([P, S], BF16, name=f"cx{m}") for m in range(2)]
        _osb = [osb.tile([P, H], F32, tag="osb", name=f"ot{t}", bufs=4)
                for t in range(NQT)]

        # ones columns for the softmax denominators (Pool, idle early)
        for t in range(NQT):
            nc.gpsimd.memset(v_sb[t][:], 1.0)

        def rope_piece(m, lo, hi, src, shf, bc, bs, eng):
            cl = slice(lo, hi)
            for hb in range(2):
                o = hb * HD
                nc.gpsimd.dma_start(shf[m][o:o + 32, cl],
                                    src[m][o + 32:o + 64, cl])
                nc.gpsimd.dma_start(shf[m][o + 32:o + 64, cl],
                                    src[m][o:o + 32, cl])
            eng.scalar_tensor_tensor(
                shf[m][:, cl], shf[m][:, cl], bs[:, m:m + 1], sinTs[:, cl],
                mybir.AluOpType.add, mybir.AluOpType.mult)
            eng.scalar_tensor_tensor(
                src[m][:, cl], src[m][:, cl], bc[:, m:m + 1], cosT[:, cl],
                mybir.AluOpType.add, mybir.AluOpType.mult)
            eng.tensor_add(src[m][:, cl], src[m][:, cl], shf[m][:, cl])

        # PSUM is hand-laid-out in two persistent tiles (region-granular
        # dependency tracking): spt = scores for one q-tile, both heads;
        # cpt = ctx accumulators + transpose slot + filler/outproj banks.
        SH = WT * P + P  # 1152: score columns per head

        def attn_scores(spt, mt, qi):
            kt0 = max(0, qi - WT)
            # slot order: [diag, far?, middles...]
            kts = [qi]
            n_edge = 1
            if qi >= WT:
                kts.append(kt0)
                n_edge = 2
            kts.extend(range(kt0 + (1 if qi >= WT else 0), qi))
            nkt = len(kts)
            prb = []
            for hb in range(2):
                ho = hb * HD
                s_ps = spt[:, hb * SH:hb * SH + SH]
                for i, kt in enumerate(kts):
                    nc.tensor.matmul(
                        s_ps[:, i * P:(i + 1) * P],
                        k_sb[mt][ho:ho + HD, kt * P:(kt + 1) * P],
                        q_sb[mt][ho:ho + HD, qi * P:(qi + 1) * P],
                        start=True, stop=True)
                probs = pb.tile([P, SH], BF16, tag="pb",
                                name=f"pr{mt}{qi}{hb}")
                nc.scalar.activation(
                    probs[:, 0:nkt * P], s_ps[:, 0:nkt * P],
                    mybir.ActivationFunctionType.Exp,
                    scale=float(1.0 / np.sqrt(HD)))
                # edge masks on Pool: DVE is near-saturated in pass B
                nc.gpsimd.tensor_mul(
                    probs[:, 0:n_edge * P], probs[:, 0:n_edge * P],
                    consts[:, 0:n_edge * P])
                prb.append(probs)
            return kts, n_edge, prb

        def attn_ctx(cxp, mt, qi, kts, n_edge, prb):
            nkt = len(kts)
            issue = list(range(n_edge, nkt)) + list(range(n_edge))
            for hb in range(2):
                h = mt * 2 + hb
                for j, i in enumerate(issue):
                    nc.tensor.matmul(
                        cxp[:, hb * VW:(hb + 1) * VW],
                        prb[hb][:, i * P:(i + 1) * P],
                        v_sb[kts[i]][:, h * VW:(h + 1) * VW],
                        start=(j == 0), stop=(j == nkt - 1))

        def norm_t(cxp, tpv, mt, qi):
            rcp = nr.tile([P, 2], F32, tag="rcp", name=f"rc{mt}{qi}")
            nrm = nr.tile([P, P], BF16, tag="nrm", name=f"nm{mt}{qi}")
            cxv = cxp.rearrange("p (h w) -> p h w", h=2)
            nc.vector.reciprocal(rcp[:], cxv[:, :, HD])
            for hb in range(2):
                nc.vector.tensor_scalar_mul(
                    nrm[:, hb * HD:(hb + 1) * HD], cxv[:, hb, 0:HD],
                    rcp[:, hb:hb + 1])
            nc.tensor.transpose(tpv, nrm[:], consts[:, 2 * P:3 * P])
            nc.vector.tensor_copy(ctx_sb[mt][:, qi * P:(qi + 1) * P], tpv)

        def vchain(ps, t):
            for c in range(CH):
                nc.tensor.matmul(ps[:, 0:GD], xc(c)[:, t * P:(t + 1) * P],
                                 wc(wv_sb, c), start=(c == 0), stop=(c == CH - 1))
            vdst = v_sb[t].rearrange("p (h d) -> p h d", h=HG)[:, :, 0:HD]
            vsrc = ps[:, 0:GD].rearrange("p (h d) -> p h d", h=HG)
            nc.scalar.copy(vdst, vsrc)

        def m1chain(ps, w_sb, dest, n):
            for c in range(CH):
                nc.tensor.matmul(
                    ps[:], wc(w_sb, c)[:, P:2 * P],
                    xc(c)[:, n * 512:(n + 1) * 512],
                    start=(c == 0), stop=(c == CH - 1))
            nc.vector.tensor_copy(dest[1][:, n * 512:(n + 1) * 512], ps[:])

        def outproj_t(fla, flb, t, last=False):
            # GPSIMD has no PSUM port and DMA can't read PSUM: drains on DVE.
            o_sb = _osb[t]
            for n, ps in enumerate((fla, flb)):
                for c in range(2):
                    nc.tensor.matmul(
                        ps[:], ctx_sb[c][:, t * P:(t + 1) * P],
                        wc(wo_sb, c, H)[:, n * 512:(n + 1) * 512],
                        start=(c == 0), stop=(c == 1))
                nc.vector.tensor_copy(o_sb[:, n * 512:(n + 1) * 512], ps[:])
                if last:
                    # halves stream out as soon as staged
                    nc.sync.dma_start(out_d.ap()[t * P:(t + 1) * P,
                                                 n * 512:(n + 1) * 512],
                                      o_sb[:, n * 512:(n + 1) * 512])
            if not last:
                # SP queue is idle in pass B; keep store triggers off the Act
                # queue (they'd stall exps) and off Pool (SWDGE descgen)
                nc.sync.dma_start(out_d.ap()[t * P:(t + 1) * P, :], o_sb[:])

        # ---- warmup + QK m=0 projection ----
        with tc.tile_pool(name="pj", bufs=8, space="PSUM") as pj:
            # warmup borrows a pj buffer (freed by the WAR dep on bank reuse);
            # one long accumulation chain so no intermediate sems serialize it
            warm = pj.tile([P, 512], F32, tag="pj", name="warm")
            for i in range(K_WARM):
                nc.tensor.matmul(warm[:, 0:3 * P], consts[:, 0:P], consts[:],
                                 start=(i == 0), stop=(i == K_WARM - 1))
            pss = [[pj.tile([P, 512], F32, tag="pj", name=f"pj{i}{n}")
                    for n in range(4)] for i in range(2)]
            for c in range(CH):
                for i, w in enumerate((wq_sb, wk_sb)):
                    for n in range(4):
                        nc.tensor.matmul(
                            pss[i][n][:], wc(w, c)[:, 0:P],
                            xc(c)[:, n * 512:(n + 1) * 512],
                            start=(c == 0), stop=(c == CH - 1))
            # drain + rope quarter-by-quarter so attention can start after
            # the first quarter; q rope on DVE, k rope on Pool (parallel)
            for n in range(4):
                nc.scalar.copy(q_sb[0][:, n * 512:(n + 1) * 512],
                               pss[0][n][:])
                nc.scalar.copy(k_sb[0][:, n * 512:(n + 1) * 512],
                               pss[1][n][:])
                rope_piece(0, n * 512, (n + 1) * 512, q_sb, qs_sb,
                           bq_sb, bqs_sb, nc.vector)
                rope_piece(0, n * 512, (n + 1) * 512, k_sb, ks_sb,
                           bk_sb, bks_sb, nc.gpsimd)

        # ---- main phase ----
        # psum: spt 2304 f32 (5 banks) + cpt 1536 f32 (3 banks) = 8 banks
        with tc.tile_pool(name="sp", bufs=1, space="PSUM") as sp_pool, \
             tc.tile_pool(name="cp", bufs=1, space="PSUM") as cp_pool:
            spt = sp_pool.tile([P, 2 * SH], F32, name="spt")
            cpt = cp_pool.tile([P, 1536], F32, name="cpt")
            CTX = cpt[:, 0:2 * VW]
            TPV = cpt[:, 2 * VW:2 * VW + HD].bitcast(BF16)    # [128, 128]
            FLA = cpt[:, 194:706]
            FLB = cpt[:, 706:1218]
            FLC = cpt[:, 1218:1474]

            # V tiles 0-3 before pass A (ctx qi needs V k-tiles <= qi)
            vchain(FLC, 0)
            vchain(FLA[:, 0:GD], 1)
            vchain(FLB[:, 0:GD], 2)
            vchain(FLC, 3)

            # m=1 chain order feeds rope(1) halves as early as possible
            m1args = [(wq_sb, q_sb, 0), (wq_sb, q_sb, 1),
                      (wk_sb, k_sb, 0), (wk_sb, k_sb, 1),
                      (wq_sb, q_sb, 2), (wq_sb, q_sb, 3),
                      (wk_sb, k_sb, 2), (wk_sb, k_sb, 3)]
            rope1 = {1: (q_sb, qs_sb, bq_sb, bqs_sb, 0),
                     3: (k_sb, ks_sb, bk_sb, bks_sb, 0),
                     5: (q_sb, qs_sb, bq_sb, bqs_sb, 1),
                     7: (k_sb, ks_sb, bk_sb, bks_sb, 1)}

            # ---- pass A (mt=0), ascending ----
            pend = None
            for qi in range(NQT):
                cur = attn_scores(spt, 0, qi)
                if pend is not None:
                    attn_ctx(CTX, 0, qi - 1, *pend)
                    norm_t(CTX, TPV, 0, qi - 1)
                pend = cur
                # PE filler work, emitted after the critical attention ops
                if qi + 4 < NQT:
                    vchain(FLC, qi + 4)
                if qi <= 7:
                    w_sb_, dest_, n_ = m1args[qi]
                    m1chain(FLA if qi % 2 == 0 else FLB, w_sb_, dest_, n_)
                if qi in rope1:
                    src_, shf_, bc_, bs_, half_ = rope1[qi]
                    for half2 in range(2):
                        lo = half_ * 1024 + half2 * 512
                        rope_piece(1, lo, lo + 512, src_, shf_, bc_, bs_,
                                   nc.vector)
            attn_ctx(CTX, 0, NQT - 1, *pend)
            norm_t(CTX, TPV, 0, NQT - 1)

            # ---- pass B (mt=1), descending; outproj trails ----
            pend = None
            for qi in range(NQT - 1, -1, -1):
                cur = attn_scores(spt, 1, qi)
                if pend is not None:
                    attn_ctx(CTX, 1, qi + 1, *pend)
                    norm_t(CTX, TPV, 1, qi + 1)
                if qi + 2 < NQT - 1:
                    outproj_t(FLA, FLB, qi + 3)
                pend = cur
            attn_ctx(CTX, 1, 0, *pend)
            norm_t(CTX, TPV, 1, 0)
            outproj_t(FLA, FLB, 2)
            outproj_t(FLA, FLB, 1, last=True)
            outproj_t(FLA, FLB, 0, last=True)

    nc.compile()
    return nc


def _rope_tables():
    inv_freq = 1.0 / (ROPE_THETA ** (np.arange(0, HD, 2, dtype=np.float64) / HD))
    t = np.arange(MAX_POS, dtype=np.float64)
    freqs = np.outer(t, inv_freq)                       # [MAX_POS, 32]
    emb = np.concatenate([freqs, freqs], axis=-1)       # [MAX_POS, 64]
    return np.cos(emb).astype(np.float32), np.sin(emb).astype(np.float32)


def kernel(hidden_states, position_ids, wq, bq, wk, bk, wv, bv, wo, bo):
    bf16 = ml_dtypes.bfloat16
    if "nc" not in _cache:
        _cache["nc"] = _build()
    nc = _cache["nc"]

    cos_t, sin_t = _rope_tables()
    pos = np.clip(np.asarray(position_ids), 0, MAX_POS - 1).astype(np.int64)

    maskd = np.triu(np.ones((P, P), np.float32))        # k <= q (diag block)
    maskf = np.tril(np.ones((P, P), np.float32), -1)    # k > q  (far block)
    ident = np.eye(P, dtype=np.float32)
    consts = np.concatenate([maskd, maskf, ident], axis=1).astype(bf16)

    in_maps = []
    for core in range(N_CORES):
        b, g = core // HG, core % HG
        sl = slice(g * GD, (g + 1) * GD)
        cos_b = cos_t[pos[b]]                            # [S, 64]
        sin_b = sin_t[pos[b]]
        cosT = np.tile(cos_b.T, (2, 1)).astype(bf16)     # [128, S]
        sin_sgn = sin_b.T.copy()                         # [64, S]
        sin_sgn[0:32] *= -1.0
        sinTs = np.tile(sin_sgn, (2, 1)).astype(bf16)
        in_maps.append({
            "xT": np.ascontiguousarray(hidden_states[b].T).astype(bf16),
            "wqT": np.ascontiguousarray(wq[sl].T).astype(bf16),
            "wkT": np.ascontiguousarray(wk[sl].T).astype(bf16),
            "wvT": np.ascontiguousarray(wv[sl].T).astype(bf16),
            "woT": np.ascontiguousarray(wo[:, sl].T).astype(bf16),
            "cosT": cosT,
            "sinTs": sinTs,
            "bq2": np.ascontiguousarray(
                bq[sl].reshape(2, P).T).astype(np.float32),
            "bk2": np.ascontiguousarray(
                bk[sl].reshape(2, P).T).astype(np.float32),
            "bq2s": np.ascontiguousarray(
                bq[sl].reshape(2, 2, 2, 32)[:, :, ::-1].reshape(
                    2, P).T).astype(np.float32),
            "bk2s": np.ascontiguousarray(
                bk[sl].reshape(2, 2, 2, 32)[:, :, ::-1].reshape(
                    2, P).T).astype(np.float32),
            "consts": consts,
        })

    res = run_bass_kernel_spmd(nc, in_maps, core_ids=list(range(N_CORES)))

    const_off = (wo @ bv + bo).astype(np.float32)        # host-folded biases
    out = np.empty((B, S, H), dtype=np.float32)
    for b in range(B):
        acc = res.results[b * HG]["out"].astype(np.float32).copy()
        for g in range(1, HG):
            acc += res.results[b * HG + g]["out"]
        out[b] = acc + const_off[None, :]
    return out


# revision 18
# speedup vs baseline: 1.1579x; 1.1577x over previous
"""Sliding-window (banded causal) multi-head attention on 8 TRN2 NeuronCores.

Sharding: 8 cores = 2 batches x 4 head-groups (4 heads of 64 dims each).
Each core computes QKV projections for its 4 heads, RoPE, banded flash
attention (window 1024), and a partial output projection (its 256 columns
of wo). The host sums the 4 partial outputs per batch element.

Schedule (v2), built around three cost-model facts: matmul time is
out-free-size x 0.4167ns with contraction/partitions free, the PE ramps
to full clock only after 3us of continuous execution, and exp on the
Activation engine (~0.83ns/col + 185ns/instr) is the attention-phase
co-bottleneck with ~58us of mandatory work.

  - warmup matmuls on the masks tile keep the PE busy from ~0.4us so the
    p-state is fully ramped when the real projections start
  - phases are interleaved so Activation's exp stream starts early:
    QK proj (m=0) -> V tiles 0-3 -> attention pass A (mt=0) with the
    remaining V tiles and the m=1 QK projection woven in as PE filler,
    -> attention pass B (mt=1, descending qi) with the output projection
    and stores trailing two tiles behind
  - ctx matmuls use probs as the stationary operand: out [q,65] per
    (head, k-tile) costs 65 columns instead of 128 (the [65,q]
    orientation wastes half the PE partitions); a PE transpose per
    (mt, q-tile) restores the [dims, tokens] layout for the out proj
  - softmax denominators ride a ones-column in V; normalization is two
    per-partition scalar multiplies on DVE; biases ride the RoPE muls
  - pass B runs q-tiles descending so the tail drains through tile 0,
    whose 1-block attention chain is the shortest
"""

from contextlib import ExitStack

import numpy as np
import ml_dtypes

import concourse.bass as bass
import concourse.tile as tile
from concourse import bacc, mybir
from concourse.bass_utils import run_bass_kernel_spmd

BF16 = mybir.dt.bfloat16
F32 = mybir.dt.float32

B, S, H = 2, 2048, 1024
NH, HD = 16, 64
WINDOW = 1024
ROPE_THETA = 10000.0
MAX_POS = 2048
N_CORES = 8
HG = 4                      # heads per core
GD = HG * HD                # 256: head-group dim per core
P = 128
NQT = S // P                # 16 q tiles
WT = WINDOW // P            # 8: window in tiles
CH = H // P                 # 8 contraction chunks
VW = HD + 1                 # 65: V width with ones column
K_WARM = 10                 # PE warmup matmuls (p-state ramp during DMA)

_cache = {}


def _build():
    nc = bacc.Bacc("TRN2", target_bir_lowering=False, debug=False,
                   enable_asserts=False, num_devices=N_CORES)

    xT_d = nc.dram_tensor("xT", [H, S], BF16, kind="ExternalInput")
    wqT_d = nc.dram_tensor("wqT", [H, GD], BF16, kind="ExternalInput")
    wkT_d = nc.dram_tensor("wkT", [H, GD], BF16, kind="ExternalInput")
    wvT_d = nc.dram_tensor("wvT", [H, GD], BF16, kind="ExternalInput")
    woT_d = nc.dram_tensor("woT", [GD, H], BF16, kind="ExternalInput")
    cosT_d = nc.dram_tensor("cosT", [P, S], BF16, kind="ExternalInput")
    sinTs_d = nc.dram_tensor("sinTs", [P, S], BF16, kind="ExternalInput")
    bq_d = nc.dram_tensor("bq2", [P, 2], F32, kind="ExternalInput")
    bk_d = nc.dram_tensor("bk2", [P, 2], F32, kind="ExternalInput")
    bqs_d = nc.dram_tensor("bq2s", [P, 2], F32, kind="ExternalInput")
    bks_d = nc.dram_tensor("bk2s", [P, 2], F32, kind="ExternalInput")
    # [diag mask | far mask | identity], bf16
    consts_d = nc.dram_tensor("consts", [P, 3 * P], BF16, kind="ExternalInput")
    out_d = nc.dram_tensor("out", [S, H], F32, kind="ExternalOutput")

    with tile.TileContext(nc) as tc, ExitStack() as ctx:
        const = ctx.enter_context(tc.tile_pool(name="const", bufs=1))
        qk = ctx.enter_context(tc.tile_pool(name="qk", bufs=1))
        vp = ctx.enter_context(tc.tile_pool(name="vp", bufs=1))
        pb = ctx.enter_context(tc.tile_pool(name="pb", bufs=4))
        nr = ctx.enter_context(tc.tile_pool(name="nr", bufs=3))
        cxs = ctx.enter_context(tc.tile_pool(name="cxs", bufs=1))
        osb = ctx.enter_context(tc.tile_pool(name="osb", bufs=4))

        wq_sb = const.tile([P, CH * GD], BF16, name="wq_sb")
        wk_sb = const.tile([P, CH * GD], BF16, name="wk_sb")
        wv_sb = const.tile([P, CH * GD], BF16, name="wv_sb")
        wo_sb = const.tile([P, 2 * H], BF16, name="wo_sb")
        x_sb = const.tile([P, CH * S], BF16, name="x_sb")
        cosT = const.tile([P, S], BF16, name="cosT")
        sinTs = const.tile([P, S], BF16, name="sinTs")
        bq_sb = const.tile([P, 2], F32, name="bq_sb")
        bk_sb = const.tile([P, 2], F32, name="bk_sb")
        bqs_sb = const.tile([P, 2], F32, name="bqs_sb")
        bks_sb = const.tile([P, 2], F32, name="bks_sb")
        consts = const.tile([P, 3 * P], BF16, name="consts")

        def chunked(dram, w):
            return dram.ap().rearrange("(c p) w -> p c w", p=P)

        # x chunks alternate between the two queues so the arrival cadence
        # (~1.3us/chunk effective) outpaces the m=0 projection's consumption
        xv = x_sb.rearrange("p (c w) -> p c w", c=CH)
        xs = chunked(xT_d, S)
        nc.sync.dma_start(consts[:], consts_d.ap())
        nc.scalar.dma_start(wq_sb.rearrange("p (c w) -> p c w", c=CH),
                            chunked(wqT_d, GD))
        nc.scalar.dma_start(wk_sb.rearrange("p (c w) -> p c w", c=CH),
                            chunked(wkT_d, GD))
        for c in range(0, CH, 2):
            nc.sync.dma_start(xv[:, c:c + 1], xs[:, c:c + 1])
        for c in range(1, CH, 2):
            nc.scalar.dma_start(xv[:, c:c + 1], xs[:, c:c + 1])
        nc.scalar.dma_start(wv_sb.rearrange("p (c w) -> p c w", c=CH),
                            chunked(wvT_d, GD))
        nc.sync.dma_start(cosT[:], cosT_d.ap())
        nc.sync.dma_start(sinTs[:], sinTs_d.ap())
        nc.sync.dma_start(bq_sb[:], bq_d.ap())
        nc.sync.dma_start(bk_sb[:], bk_d.ap())
        nc.sync.dma_start(bqs_sb[:], bqs_d.ap())
        nc.sync.dma_start(bks_sb[:], bks_d.ap())
        nc.scalar.dma_start(wo_sb.rearrange("p (c w) -> p c w", c=2),
                            chunked(woT_d, H))

        def xc(c):
            return x_sb[:, c * S:(c + 1) * S]

        def wc(w_sb, c, width=GD):
            return w_sb[:, c * width:(c + 1) * width]

        q_sb = [qk.tile([P, S], BF16, name=f"q{m}") for m in range(2)]
        k_sb = [qk.tile([P, S], BF16, name=f"k{m}") for m in range(2)]
        qs_sb = [qk.tile([P, S], BF16, name=f"qs{m}") for m in range(2)]
        ks_sb = [qk.tile([P, S], BF16, name=f"ks{m}") for m in range(2)]
        v_sb = [vp.tile([P, HG * VW], BF16, name=f"v{t}") for t in range(NQT)]
        ctx_sb = [cxs.tile([P, S], BF16, name=f"cx{m}") for m in range(2)]
        _osb = [osb.tile([P, H], F32, tag="osb", name=f"ot{t}", bufs=4)
                for t in range(NQT)]

        # ones columns for the softmax denominators (Pool, idle early)
        for t in range(NQT):
            nc.gpsimd.memset(v_sb[t][:], 1.0)

        def rope_piece(m, lo, hi, src, shf, bc, bs, eng):
            cl = slice(lo, hi)
            for hb in range(2):
                o = hb * HD
                nc.sync.dma_start(shf[m][o:o + 32, cl],
                                  src[m][o + 32:o + 64, cl])
                nc.sync.dma_start(shf[m][o + 32:o + 64, cl],
                                  src[m][o:o + 32, cl])
            eng.scalar_tensor_tensor(
                shf[m][:, cl], shf[m][:, cl], bs[:, m:m + 1], sinTs[:, cl],
                mybir.AluOpType.add, mybir.AluOpType.mult)
            eng.scalar_tensor_tensor(
                src[m][:, cl], src[m][:, cl], bc[:, m:m + 1], cosT[:, cl],
                mybir.AluOpType.add, mybir.AluOpType.mult)
            eng.tensor_add(src[m][:, cl], src[m][:, cl], shf[m][:, cl])

        # PSUM is hand-laid-out in two persistent tiles (region-granular
        # dependency tracking): spt = scores for one q-tile, both heads;
        # cpt = ctx accumulators + transpose slot + filler/outproj banks.
        SH = WT * P + P  # 1152: score columns per head

        def attn_scores(spt, mt, qi):
            kt0 = max(0, qi - WT)
            # slot order: [diag, far?, middles...]
            kts = [qi]
            n_edge = 1
            if qi >= WT:
                kts.append(kt0)
                n_edge = 2
            kts.extend(range(kt0 + (1 if qi >= WT else 0), qi))
            nkt = len(kts)
            prb = []
            for hb in range(2):
                ho = hb * HD
                s_ps = spt[:, hb * SH:hb * SH + SH]
                for i, kt in enumerate(kts):
                    nc.tensor.matmul(
                        s_ps[:, i * P:(i + 1) * P],
                        k_sb[mt][ho:ho + HD, kt * P:(kt + 1) * P],
                        q_sb[mt][ho:ho + HD, qi * P:(qi + 1) * P],
                        start=True, stop=True)
                probs = pb.tile([P, SH], BF16, tag="pb",
                                name=f"pr{mt}{qi}{hb}")
                nc.scalar.activation(
                    probs[:, 0:nkt * P], s_ps[:, 0:nkt * P],
                    mybir.ActivationFunctionType.Exp,
                    scale=float(1.0 / np.sqrt(HD)))
                # edge masks on Pool: DVE is near-saturated in pass B
                nc.gpsimd.tensor_mul(
                    probs[:, 0:n_edge * P], probs[:, 0:n_edge * P],
                    consts[:, 0:n_edge * P])
                prb.append(probs)
            return kts, n_edge, prb

        def attn_ctx(cxp, mt, qi, kts, n_edge, prb):
            nkt = len(kts)
            issue = list(range(n_edge, nkt)) + list(range(n_edge))
            for hb in range(2):
                h = mt * 2 + hb
                for j, i in enumerate(issue):
                    nc.tensor.matmul(
                        cxp[:, hb * VW:(hb + 1) * VW],
                        prb[hb][:, i * P:(i + 1) * P],
                        v_sb[kts[i]][:, h * VW:(h + 1) * VW],
                        start=(j == 0), stop=(j == nkt - 1))

        def norm_t(cxp, tpv, mt, qi):
            rcp = nr.tile([P, 2], F32, tag="rcp", name=f"rc{mt}{qi}")
            nrm = nr.tile([P, P], BF16, tag="nrm", name=f"nm{mt}{qi}")
            cxv = cxp.rearrange("p (h w) -> p h w", h=2)
            nc.vector.reciprocal(rcp[:], cxv[:, :, HD])
            for hb in range(2):
                nc.vector.tensor_scalar_mul(
                    nrm[:, hb * HD:(hb + 1) * HD], cxv[:, hb, 0:HD],
                    rcp[:, hb:hb + 1])
            nc.tensor.transpose(tpv, nrm[:], consts[:, 2 * P:3 * P])
            nc.vector.tensor_copy(ctx_sb[mt][:, qi * P:(qi + 1) * P], tpv)

        def vchain(ps, t):
            for c in range(CH):
                nc.tensor.matmul(ps[:, 0:GD], xc(c)[:, t * P:(t + 1) * P],
                                 wc(wv_sb, c), start=(c == 0), stop=(c == CH - 1))
            vdst = v_sb[t].rearrange("p (h d) -> p h d", h=HG)[:, :, 0:HD]
            vsrc = ps[:, 0:GD].rearrange("p (h d) -> p h d", h=HG)
            nc.scalar.copy(vdst, vsrc)

        def m1chain(ps, w_sb, dest, n):
            for c in range(CH):
                nc.tensor.matmul(
                    ps[:], wc(w_sb, c)[:, P:2 * P],
                    xc(c)[:, n * 512:(n + 1) * 512],
                    start=(c == 0), stop=(c == CH - 1))
            nc.vector.tensor_copy(dest[1][:, n * 512:(n + 1) * 512], ps[:])

        def outproj_t(fla, flb, t, last=False):
            # GPSIMD has no PSUM port and DMA can't read PSUM: drains on DVE.
            o_sb = _osb[t]
            for n, ps in enumerate((fla, flb)):
                for c in range(2):
                    nc.tensor.matmul(
                        ps[:], ctx_sb[c][:, t * P:(t + 1) * P],
                        wc(wo_sb, c, H)[:, n * 512:(n + 1) * 512],
                        start=(c == 0), stop=(c == 1))
                nc.vector.tensor_copy(o_sb[:, n * 512:(n + 1) * 512], ps[:])
                if last:
                    # halves stream out as soon as staged
                    nc.sync.dma_start(out_d.ap()[t * P:(t + 1) * P,
                                                 n * 512:(n + 1) * 512],
                                      o_sb[:, n * 512:(n + 1) * 512])
            if not last:
                # SP queue is idle in pass B; keep store triggers off the Act
                # queue (they'd stall exps) and off Pool (SWDGE descgen)
                nc.sync.dma_start(out_d.ap()[t * P:(t + 1) * P, :], o_sb[:])

        # ---- warmup + QK m=0 projection ----
        with tc.tile_pool(name="pj", bufs=8, space="PSUM") as pj:
            # warmup borrows a pj buffer (freed by the WAR dep on bank reuse);
            # one long accumulation chain so no intermediate sems serialize it
            warm = pj.tile([P, 512], F32, tag="pj", name="warm")
            for i in range(K_WARM):
                nc.tensor.matmul(warm[:, 0:3 * P], consts[:, 0:P], consts[:],
                                 start=(i == 0), stop=(i == K_WARM - 1))
            pss = [[pj.tile([P, 512], F32, tag="pj", name=f"pj{i}{n}")
                    for n in range(4)] for i in range(2)]
            for c in range(CH):
                for i, w in enumerate((wq_sb, wk_sb)):
                    for n in range(4):
                        nc.tensor.matmul(
                            pss[i][n][:], wc(w, c)[:, 0:P],
                            xc(c)[:, n * 512:(n + 1) * 512],
                            start=(c == 0), stop=(c == CH - 1))
            # drain + rope quarter-by-quarter so attention can start after
            # the first quarter; q rope on DVE, k rope on Pool (parallel)
            for n in range(4):
                nc.scalar.copy(q_sb[0][:, n * 512:(n + 1) * 512],
                               pss[0][n][:])
                nc.scalar.copy(k_sb[0][:, n * 512:(n + 1) * 512],
                               pss[1][n][:])
                rope_piece(0, n * 512, (n + 1) * 512, q_sb, qs_sb,
                           bq_sb, bqs_sb, nc.vector)
                rope_piece(0, n * 512, (n + 1) * 512, k_sb, ks_sb,
                           bk_sb, bks_sb, nc.gpsimd)

        # ---- main phase ----
        # psum: spt 2304 f32 (5 banks) + cpt 1536 f32 (3 banks) = 8 banks
        with tc.tile_pool(name="sp", bufs=1, space="PSUM") as sp_pool, \
             tc.tile_pool(name="cp", bufs=1, space="PSUM") as cp_pool:
            spt = sp_pool.tile([P, 2 * SH], F32, name="spt")
            cpt = cp_pool.tile([P, 1536], F32, name="cpt")
            CTX = cpt[:, 0:2 * VW]
            TPV = cpt[:, 2 * VW:2 * VW + HD].bitcast(BF16)    # [128, 128]
            FLA = cpt[:, 194:706]
            FLB = cpt[:, 706:1218]
            FLC = cpt[:, 1218:1474]

            # V tiles 0-3 before pass A (ctx qi needs V k-tiles <= qi)
            vchain(FLC, 0)
            vchain(FLA[:, 0:GD], 1)
            vchain(FLB[:, 0:GD], 2)
            vchain(FLC, 3)

            # m=1 chain order feeds rope(1) halves as early as possible
            m1args = [(wq_sb, q_sb, 0), (wq_sb, q_sb, 1),
                      (wk_sb, k_sb, 0), (wk_sb, k_sb, 1),
                      (wq_sb, q_sb, 2), (wq_sb, q_sb, 3),
                      (wk_sb, k_sb, 2), (wk_sb, k_sb, 3)]
            rope1 = {1: (q_sb, qs_sb, bq_sb, bqs_sb, 0),
                     3: (k_sb, ks_sb, bk_sb, bks_sb, 0),
                     5: (q_sb, qs_sb, bq_sb, bqs_sb, 1),
                     7: (k_sb, ks_sb, bk_sb, bks_sb, 1)}

            # ---- pass A (mt=0), ascending ----
            pend = None
            for qi in range(NQT):
                cur = attn_scores(spt, 0, qi)
                if pend is not None:
                    attn_ctx(CTX, 0, qi - 1, *pend)
                    norm_t(CTX, TPV, 0, qi - 1)
                pend = cur
                # PE filler work, emitted after the critical attention ops
                if qi + 4 < NQT:
                    vchain(FLC, qi + 4)
                if qi <= 7:
                    w_sb_, dest_, n_ = m1args[qi]
                    m1chain(FLA if qi % 2 == 0 else FLB, w_sb_, dest_, n_)
                if qi in rope1:
                    src_, shf_, bc_, bs_, half_ = rope1[qi]
                    for half2 in range(2):
                        lo = half_ * 1024 + half2 * 512
                        rope_piece(1, lo, lo + 512, src_, shf_, bc_, bs_,
                                   nc.vector)
            attn_ctx(CTX, 0, NQT - 1, *pend)
            norm_t(CTX, TPV, 0, NQT - 1)

            # ---- pass B (mt=1), descending; outproj trails ----
            pend = None
            for qi in range(NQT - 1, -1, -1):
                cur = attn_scores(spt, 1, qi)
                if pend is not None:
                    attn_ctx(CTX, 1, qi + 1, *pend)
                    norm_t(CTX, TPV, 1, qi + 1)
                if qi + 2 < NQT - 1:
                    outproj_t(FLA, FLB, qi + 3)
                pend = cur
            attn_ctx(CTX, 1, 0, *pend)
            norm_t(CTX, TPV, 1, 0)
            outproj_t(FLA, FLB, 2)
            outproj_t(FLA, FLB, 1, last=True)
            outproj_t(FLA, FLB, 0, last=True)

    nc.compile()
    return nc


def _rope_tables():
    inv_freq = 1.0 / (ROPE_THETA ** (np.arange(0, HD, 2, dtype=np.float64) / HD))
    t = np.arange(MAX_POS, dtype=np.float64)
    freqs = np.outer(t, inv_freq)                       # [MAX_POS, 32]
    emb = np.concatenate([freqs, freqs], axis=-1)       # [MAX_POS, 64]
    return np.cos(emb).astype(np.float32), np.sin(emb).astype(np.float32)


def kernel(hidden_states, position_ids, wq, bq, wk, bk, wv, bv, wo, bo):
    bf16 = ml_dtypes.bfloat16
    if "nc" not in _cache:
        _cache["nc"] = _build()
    nc = _cache["nc"]

    cos_t, sin_t = _rope_tables()
    pos = np.clip(np.asarray(position_ids), 0, MAX_POS - 1).astype(np.int64)

    maskd = np.triu(np.ones((P, P), np.float32))        # k <= q (diag block)
    maskf = np.tril(np.ones((P, P), np.float32), -1)    # k > q  (far block)
    ident = np.eye(P, dtype=np.float32)
    consts = np.concatenate([maskd, maskf, ident], axis=1).astype(bf16)

    in_maps = []
    for core in range(N_CORES):
        b, g = core // HG, core % HG
        sl = slice(g * GD, (g + 1) * GD)
        cos_b = cos_t[pos[b]]                            # [S, 64]
        sin_b = sin_t[pos[b]]
        cosT = np.tile(cos_b.T, (2, 1)).astype(bf16)     # [128, S]
        sin_sgn = sin_b.T.copy()                         # [64, S]
        sin_sgn[0:32] *= -1.0
        sinTs = np.tile(sin_sgn, (2, 1)).astype(bf16)
        in_maps.append({
            "xT": np.ascontiguousarray(hidden_states[b].T).astype(bf16),
            "wqT": np.ascontiguousarray(wq[sl].T).astype(bf16),
            "wkT": np.ascontiguousarray(wk[sl].T).astype(bf16),
            "wvT": np.ascontiguousarray(wv[sl].T).astype(bf16),
            "woT": np.ascontiguousarray(wo[:, sl].T).astype(bf16),
            "cosT": cosT,
            "sinTs": sinTs,
            "bq2": np.ascontiguousarray(
                bq[sl].reshape(2, P).T).astype(np.float32),
            "bk2": np.ascontiguousarray(
                bk[sl].reshape(2, P).T).astype(np.float32),
            "bq2s": np.ascontiguousarray(
                bq[sl].reshape(2, 2, 2, 32)[:, :, ::-1].reshape(
                    2, P).T).astype(np.float32),
            "bk2s": np.ascontiguousarray(
                bk[sl].reshape(2, 2, 2, 32)[:, :, ::-1].reshape(
                    2, P).T).astype(np.float32),
            "consts": consts,
        })

    res = run_bass_kernel_spmd(nc, in_maps, core_ids=list(range(N_CORES)))

    const_off = (wo @ bv + bo).astype(np.float32)        # host-folded biases
    out = np.empty((B, S, H), dtype=np.float32)
    for b in range(B):
        acc = res.results[b * HG]["out"].astype(np.float32).copy()
        for g in range(1, HG):
            acc += res.results[b * HG + g]["out"]
        out[b] = acc + const_off[None, :]
    return out
